# revision 1
# baseline (speedup 1.0000x reference)
"""Trainium2 Bass kernel for the CustomGNNLayer problem.

Strategy (data-parallel over Q, 8 queries/core on 8 cores):
  host: gather hs rows, transpose layouts, compact node slots per (q,k) group
        (drop all-zero padded slots; pad kept counts to PAD_MULT classes with a
        per-block class profile uniform across cores so one SPMD program fits
        all cores), build one-hot prob-gather matrices and fold mask / mean
        divisors into a mask-factor tensor.
  device (per core): classification softmax + one-hot prob gather; gq = tanh
        projection; per (q,k) block: X^T = Wn^T @ nodesT (f32r matmuls), tanh
        on ScalarE, dots = gq . tanhX via PE, scatter to [N,M] buffer prefilled
        with the all-zero-slot dot value c_q, group softmax + global softmax,
        weighted sum of nodes via PE-broadcast wa + fused DVE multiply-reduce,
        final tanh projection -> updated rows.
  host: res = hidden_states.copy(); res[gnn_idx] += rows.
"""
import sys

sys.path.insert(0, "/opt/trn_rl_repo")

import numpy as np

import concourse.bacc as bacc
import concourse.bass as bass
import concourse.tile as tile
from concourse import mybir
from concourse.bass_utils import run_bass_kernel_spmd

F32 = mybir.dt.float32
F32R = mybir.dt.float32r
AF = mybir.ActivationFunctionType
ALU = mybir.AluOpType
AX = mybir.AxisListType

Q, K, N, M = 64, 2, 32, 64
E, D, R, S = 256, 1024, 200, 8192
NCORES = 8
QPC = Q // NCORES          # 8 queries per core
NB = QPC * K               # 16 blocks per core, b = qi*K + k
PAD_MULT = 8
CHUNK = 512
ET = E // 128              # 2 e-tiles
DT = D // 128              # 8 d-tiles
KT = D // 128              # 8 k-tiles for D-contraction


def _chunks(s):
    n = (s + CHUNK - 1) // CHUNK
    h = s // 2
    base, rem = divmod(h, n)
    sizes = [2 * (base + (1 if i < rem else 0)) for i in range(n)]
    out, off = [], 0
    for sz in sizes:
        out.append((off, sz))
        off += sz
    return out


def _host_prep(inputs):
    hs = np.ascontiguousarray(inputs["hidden_states"], dtype=np.float32)
    nodes = np.ascontiguousarray(inputs["nodes"], dtype=np.float32)
    prob_idx = np.asarray(inputs["prob_idx"])
    gnn_idx = np.asarray(inputs["gnn_idx"]).astype(np.int64)
    rel_idx = np.asarray(inputs["rel_idx"]).astype(np.int64)

    nz = np.any(nodes != 0.0, axis=4)          # [Q,K,N,M] kept slots
    lens = nz.sum(axis=3)                      # [Q,K,N]
    Lg = np.minimum(((np.maximum(lens, 1) + PAD_MULT - 1) // PAD_MULT) * PAD_MULT, M)

    # per-block-index profile: position-wise max of descending-sorted Lg across cores
    profiles = []   # [NB][N] descending class sizes, uniform across cores
    for qi in range(QPC):
        for k in range(K):
            seqs = [np.sort(Lg[c * QPC + qi, k])[::-1] for c in range(NCORES)]
            profiles.append(np.max(np.stack(seqs), axis=0))
    S_b = [int(p.sum()) for p in profiles]
    segs = []       # [NB] list of (L, row0, cnt, slot_off)
    for p in profiles:
        s, off, r0 = [], 0, 0
        i = 0
        while i < N:
            j = i
            while j < N and p[j] == p[i]:
                j += 1
            L = int(p[i])
            s.append((L, i, j - i, off))
            off += L * (j - i)
            i = j
        segs.append(s)

    mask0 = (nodes[..., 0] != 0.0)             # [Q,K,N,M] reference mask

    per_core = []
    for c in range(NCORES):
        qs = np.arange(c * QPC, (c + 1) * QPC)
        nt_flat = np.empty(sum(2 * 128 * s for s in S_b), np.float32)
        maskf = np.zeros((NB, N, M), np.float32)
        onehot = np.zeros((NB, R, N), np.float32)
        ntoff = 0
        for qi in range(QPC):
            q = qs[qi]
            for k in range(K):
                b = qi * K + k
                prof = profiles[b]
                order = np.argsort(-Lg[q, k], kind="stable")   # ranks -> groups
                comp = np.zeros((S_b[b], E), np.float32)
                off = 0
                for rank, g in enumerate(order):
                    L = int(prof[rank])
                    keep = np.nonzero(nz[q, k, g])[0]
                    nkeep = len(keep)
                    comp[off : off + nkeep] = nodes[q, k, g, keep]
                    maskf[b, rank, :nkeep] = mask0[q, k, g, keep].astype(np.float32)
                    onehot[b, prob_idx[q, k, g], rank] = 1.0
                    off += L
                nt = comp.T                                    # [E, S_b]
                sz = 2 * 128 * S_b[b]
                nt_flat[ntoff : ntoff + sz] = nt.reshape(-1)
                ntoff += sz
        maskf *= 1.0 / (N * M * K)
        per_core.append({
            "nodesT": nt_flat,
            "hsrelT": np.ascontiguousarray(hs[rel_idx[qs]].T),
            "hsgnnT": np.ascontiguousarray(hs[gnn_idx[qs]].T),
            "maskf": maskf,
            "onehot": onehot,
        })

    shared = {
        "Wc": np.ascontiguousarray(inputs["Wc"], dtype=np.float32),
        "Wq": np.ascontiguousarray(inputs["Wq"], dtype=np.float32),
        "Wn": np.ascontiguousarray(inputs["Wn"], dtype=np.float32),
        "Wg": np.ascontiguousarray(inputs["Wg"], dtype=np.float32),
        "bc": np.ascontiguousarray(inputs["bc"], dtype=np.float32),
        "bq": np.ascontiguousarray(np.asarray(inputs["bq"], np.float32).reshape(8, 128).T),
        "bn": np.ascontiguousarray(np.asarray(inputs["bn"], np.float32).reshape(8, 128).T),
        "bg": np.ascontiguousarray(np.asarray(inputs["bg"], np.float32).reshape(8, 128).T),
        "id8": np.eye(8, dtype=np.float32),
        "ones128": np.ones((1, 128), np.float32),
    }
    for pc in per_core:
        pc.update(shared)
    return per_core, S_b, segs, gnn_idx, hs


def _build_program(S_b, segs):
    import os
    STAGE = int(os.environ.get("K_STAGE", "7"))
    nc = bacc.Bacc("TRN2", target_bir_lowering=False, debug=False,
                   num_devices=NCORES)
    S_MAX = max(S_b)
    NT_TOT = sum(2 * 128 * s for s in S_b)

    d_nodesT = nc.dram_tensor("nodesT", [NT_TOT], F32R, kind="ExternalInput").ap()
    d_hsrelT = nc.dram_tensor("hsrelT", [D, QPC], F32R, kind="ExternalInput").ap()
    d_hsgnnT = nc.dram_tensor("hsgnnT", [D, QPC], F32R, kind="ExternalInput").ap()
    d_Wc = nc.dram_tensor("Wc", [D, R], F32R, kind="ExternalInput").ap()
    d_Wq = nc.dram_tensor("Wq", [D, D], F32R, kind="ExternalInput").ap()
    d_Wn = nc.dram_tensor("Wn", [E, D], F32R, kind="ExternalInput").ap()
    d_Wg = nc.dram_tensor("Wg", [E, D], F32R, kind="ExternalInput").ap()
    d_bc = nc.dram_tensor("bc", [R], F32, kind="ExternalInput").ap()
    d_bq = nc.dram_tensor("bq", [128, KT], F32, kind="ExternalInput").ap()
    d_bn = nc.dram_tensor("bn", [128, KT], F32, kind="ExternalInput").ap()
    d_bg = nc.dram_tensor("bg", [128, KT], F32, kind="ExternalInput").ap()
    d_id8 = nc.dram_tensor("id8", [8, 8], F32, kind="ExternalInput").ap()
    d_ones = nc.dram_tensor("ones128", [1, 128], F32R, kind="ExternalInput").ap()
    d_maskf = nc.dram_tensor("maskf", [NB, N, M], F32, kind="ExternalInput").ap()
    d_onehot = nc.dram_tensor("onehot", [NB, R, N], F32R, kind="ExternalInput").ap()
    d_outT = nc.dram_tensor("outT", [D, QPC], F32, kind="ExternalOutput").ap()

    # DRAM scratch
    d_dots = nc.dram_tensor("sc_dots", [NB, 2048], F32).ap()
    d_wa = nc.dram_tensor("sc_wa", [NB, 2048], F32R).ap()
    d_ginv = nc.dram_tensor("sc_ginv", [NB, 1], F32).ap()
    d_cq = nc.dram_tensor("sc_cq", [QPC, 1], F32).ap()

    with tile.TileContext(nc) as tc:
        with tc.tile_pool(name="wts", bufs=1) as wts, \
             tc.tile_pool(name="big", bufs=2) as big, \
             tc.tile_pool(name="strm", bufs=4) as strm, \
             tc.tile_pool(name="sml", bufs=4) as sml, \
             tc.tile_pool(name="ps", bufs=3, space="PSUM") as ps, \
             tc.tile_pool(name="psd", bufs=2, space="PSUM") as psd, \
             tc.tile_pool(name="psw", bufs=2, space="PSUM") as psw:

            # ---------------- load constants ----------------
            sWc = wts.tile([128, KT, R], F32R)
            nc.sync.dma_start(sWc, d_Wc.rearrange("(t p) r -> p t r", p=128))
            sWq = wts.tile([128, KT, D], F32R)
            nc.sync.dma_start(sWq, d_Wq.rearrange("(t p) r -> p t r", p=128))
            sWn = wts.tile([128, ET, D], F32R)
            nc.sync.dma_start(sWn, d_Wn.rearrange("(t p) r -> p t r", p=128))
            sWg = wts.tile([128, ET, D], F32R)
            nc.sync.dma_start(sWg, d_Wg.rearrange("(t p) r -> p t r", p=128))
            sRelT = wts.tile([128, KT, QPC], F32R)
            nc.sync.dma_start(sRelT, d_hsrelT.rearrange("(t p) q -> p t q", p=128))
            sGnnT = wts.tile([128, KT, QPC], F32R)
            nc.sync.dma_start(sGnnT, d_hsgnnT.rearrange("(t p) q -> p t q", p=128))
            sbq = wts.tile([128, KT], F32)
            nc.sync.dma_start(sbq, d_bq)
            sbn = wts.tile([128, KT], F32)
            nc.sync.dma_start(sbn, d_bn)
            sbg = wts.tile([128, KT], F32)
            nc.sync.dma_start(sbg, d_bg)
            sid8 = wts.tile([8, 8], F32)
            nc.sync.dma_start(sid8, d_id8)
            sones = wts.tile([1, 128], F32R)
            nc.sync.dma_start(sones, d_ones)
            sbc = wts.tile([QPC, R], F32)
            nc.sync.dma_start(
                sbc, bass.AP(tensor=d_bc.tensor, offset=0, ap=[[0, QPC], [1, R]]))
            smaskf = wts.tile([N, NB, M], F32)
            nc.sync.dma_start(smaskf, d_maskf.rearrange("b n m -> n b m"))
            soh0 = wts.tile([128, NB, N], F32R)
            nc.sync.dma_start(soh0, d_onehot[:, 0:128, :].rearrange("b p n -> p b n"))
            soh1 = wts.tile([128, NB, N], F32R)
            nc.sync.dma_start(
                soh1[0 : R - 128], d_onehot[:, 128:R, :].rearrange("b p n -> p b n"))

            # ---------------- stage 0 ----------------
            # rel_logits [QPC, R] = hsrelT^T @ Wc ; softmax*10 ; transpose
            p_rl = ps.tile([128, CHUNK], F32, tag="mm")
            for t in range(KT):
                nc.tensor.matmul(p_rl[0:QPC, 0:R], sRelT[:, t, :], sWc[:, t, :],
                                 start=(t == 0), stop=(t == KT - 1))
            t_rl = sml.tile([QPC, R], F32)
            nc.vector.tensor_tensor(t_rl, p_rl[0:QPC, 0:R], sbc, op=ALU.add)
            t_mx = sml.tile([QPC, 1], F32)
            nc.vector.tensor_reduce(t_mx, t_rl, axis=AX.X, op=ALU.max)
            t_nmx = sml.tile([QPC, 1], F32)
            nc.vector.tensor_scalar_mul(t_nmx, t_mx, -1.0)
            t_exp = sml.tile([QPC, R], F32)
            t_sum = sml.tile([QPC, 1], F32)
            nc.scalar.activation(t_exp, t_rl, AF.Exp, bias=t_nmx, scale=1.0,
                                 accum_out=t_sum)
            t_inv = sml.tile([QPC, 1], F32)
            nc.vector.reciprocal(t_inv, t_sum)
            t_rp = sml.tile([QPC, R], F32)   # rel_prob * 10
            nc.vector.tensor_scalar(t_rp, t_exp, t_inv, 10.0, op0=ALU.mult,
                                    op1=ALU.mult)
            # transpose -> rel_probT [R, QPC] (two PE transposes)
            t_rpT = sml.tile([128, 2, QPC], F32R)
            for half, (c0, cw) in enumerate(((0, 128), (128, R - 128))):
                p_tr = ps.tile([128, CHUNK], F32, tag="mm")
                nc.tensor.matmul(p_tr[0:cw, 0:QPC], t_rp[:, c0 : c0 + cw], sid8,
                                 is_transpose=True, start=True, stop=True)
                nc.vector.tensor_copy(t_rpT[0:cw, half, :], p_tr[0:cw, 0:QPC])

            # gqT [D, QPC] as [128, DT, QPC]
            t_gqT = wts.tile([128, DT, QPC], F32R)
            for mt in range(DT):
                p_gq = ps.tile([128, CHUNK], F32, tag="mm")
                for t in range(KT):
                    nc.tensor.matmul(p_gq[:, 0:QPC], sWq[:, t, mt * 128:(mt + 1) * 128],
                                     sGnnT[:, t, :], start=(t == 0), stop=(t == KT - 1))
                nc.scalar.activation(t_gqT[:, mt, :], p_gq[:, 0:QPC],
                                     AF.Tanh, bias=sbq[:, mt : mt + 1], scale=1.0)
            # tanh(bn) [D,1] as [128, DT]
            t_tbn = wts.tile([128, DT + 1], F32R)
            nc.scalar.activation(t_tbn[:, 0:DT], sbn, AF.Tanh)
            nc.scalar.activation(t_tbn[:, DT : DT + 1], sbn[:, 0:1], AF.Tanh,
                                 scale=0.0)
            # c_q [QPC, 1]
            p_cq = ps.tile([128, CHUNK], F32, tag="mm")
            for mt in range(DT):
                nc.tensor.matmul(p_cq[0:QPC, 0:2], t_gqT[:, mt, :],
                                 t_tbn[:, mt : mt + 2], start=(mt == 0),
                                 stop=(mt == DT - 1))
            t_cq = sml.tile([QPC, 1], F32)
            nc.vector.tensor_copy(t_cq, p_cq[0:QPC, 0:1])
            nc.sync.dma_start(d_cq, t_cq)

            # probs10 columns per block [N, 1]
            t_pr = wts.tile([N, NB], F32)
            for b in range(NB):
                qi = b // K
                q0 = qi if qi < QPC - 1 else qi - 1
                col = qi - q0
                p_pb = ps.tile([128, CHUNK], F32, tag="mm")
                nc.tensor.matmul(p_pb[0:N, 0:2], soh0[:, b, :],
                                 t_rpT[:, 0, q0 : q0 + 2],
                                 start=True, stop=False)
                nc.tensor.matmul(p_pb[0:N, 0:2], soh1[0 : R - 128, b, :],
                                 t_rpT[0 : R - 128, 1, q0 : q0 + 2],
                                 start=False, stop=True)
                nc.vector.tensor_copy(t_pr[:, b : b + 1], p_pb[0:N, col : col + 1])

            # ---------------- main loop ----------------
            if STAGE >= 6:
                t_pooled = wts.tile([128, ET, QPC], F32)
            else:
                t_pooled = None
            nt_off = 0
            from collections import defaultdict
            partials = defaultdict(list)
            for b in range(NB if STAGE >= 2 else 0):
                qi, k = b // K, b % K
                sb = S_b[b]
                chs = _chunks(sb)

                t_nt = big.tile([128, ET, S_MAX], F32R, tag="nt")
                nc.sync.dma_start(
                    t_nt[:, :, 0:sb],
                    bass.AP(tensor=d_nodesT.tensor, offset=nt_off,
                            ap=[[sb, 128], [128 * sb, ET], [1, sb]]))
                nt_off += 2 * 128 * sb

                t_dots = big.tile([1, S_MAX], F32, tag="dots")
                for (c0, cw) in chs:
                    p_dot = psd.tile([1, CHUNK], F32, tag="dot")
                    for dt_i in range(DT):
                        p_x = ps.tile([128, CHUNK], F32, tag="mm")
                        for et in range(ET):
                            nc.tensor.matmul(
                                p_x[:, 0:cw],
                                sWn[:, et, dt_i * 128:(dt_i + 1) * 128],
                                t_nt[:, et, c0 : c0 + cw],
                                start=(et == 0), stop=(et == ET - 1))
                        t_tx = strm.tile([128, CHUNK], F32R, tag="tx")
                        nc.scalar.activation(t_tx[:, 0:cw], p_x[:, 0:cw],
                                             AF.Tanh, bias=sbn[:, dt_i : dt_i + 1],
                                             scale=1.0)
                        nc.tensor.matmul(p_dot[0:1, 0:cw], t_gqT[:, dt_i, qi : qi + 1],
                                         t_tx[:, 0:cw], start=(dt_i == 0),
                                         stop=(dt_i == DT - 1))
                    nc.vector.tensor_copy(t_dots[0:1, c0 : c0 + cw], p_dot[0:1, 0:cw])
                nc.sync.dma_start(d_dots[b : b + 1, 0:sb], t_dots[0:1, 0:sb])

                if STAGE < 3:
                    continue
                # scatter into [N, M] buffer prefilled with c_q
                t_dbuf = sml.tile([N, M], F32, tag="dbuf")
                t_cqc = sml.tile([N, 1], F32, tag="cqc")
                nc.sync.dma_start(
                    t_cqc,
                    bass.AP(tensor=d_cq.tensor, offset=qi, ap=[[0, N], [1, 1]]))
                nc.vector.tensor_scalar(t_dbuf, smaskf[:, b, :], 0.0, t_cqc,
                                        op0=ALU.mult, op1=ALU.add)
                for (L, r0, cnt, soff) in segs[b]:
                    nc.sync.dma_start(
                        t_dbuf[r0 : r0 + cnt, 0:L],
                        d_dots[b, soff : soff + cnt * L].rearrange("(c l) -> c l", l=L))

                # group softmax + probs + global softmax
                t_gmx = sml.tile([N, 1], F32, tag="gmx")
                nc.vector.tensor_reduce(t_gmx, t_dbuf, axis=AX.X, op=ALU.max)
                t_gnmx = sml.tile([N, 1], F32, tag="gnmx")
                nc.vector.tensor_scalar_mul(t_gnmx, t_gmx, -1.0)
                t_ex = sml.tile([N, M], F32, tag="ex")
                t_rs = sml.tile([N, 1], F32, tag="rs")
                nc.scalar.activation(t_ex, t_dbuf, AF.Exp, bias=t_gnmx, scale=1.0,
                                     accum_out=t_rs)
                t_ri = sml.tile([N, 1], F32, tag="ri")
                nc.vector.reciprocal(t_ri, t_rs)
                t_lg = sml.tile([N, M], F32, tag="lg")
                nc.vector.tensor_scalar(t_lg, t_ex, t_ri, t_pr[:, b : b + 1],
                                        op0=ALU.mult, op1=ALU.mult)
                t_gl = sml.tile([N, M], F32, tag="gl")
                t_grs = sml.tile([N, 1], F32, tag="grs")
                nc.scalar.activation(t_gl, t_lg, AF.Exp, accum_out=t_grs)
                t_gs = sml.tile([1, 1], F32, tag="gs")
                nc.gpsimd.tensor_reduce(t_gs, t_grs, axis=AX.C, op=ALU.add)
                t_gi = sml.tile([1, 1], F32, tag="gi")
                nc.vector.reciprocal(t_gi, t_gs)
                nc.sync.dma_start(d_ginv[b : b + 1, :], t_gi)
                t_gic = sml.tile([N, 1], F32, tag="gic")
                nc.sync.dma_start(
                    t_gic,
                    bass.AP(tensor=d_ginv.tensor, offset=b, ap=[[0, N], [0, 1]]))
                t_wa = sml.tile([N, M], F32R, tag="wa")
                nc.vector.scalar_tensor_tensor(
                    t_wa, t_gl, t_gic, smaskf[:, b, :],
                    op0=ALU.mult, op1=ALU.mult)

                # gather back to compacted order
                for (L, r0, cnt, soff) in segs[b]:
                    nc.sync.dma_start(
                        d_wa[b, soff : soff + cnt * L].rearrange("(c l) -> c l", l=L),
                        t_wa[r0 : r0 + cnt, 0:L])
                t_wac = big.tile([1, S_MAX], F32R, tag="wac")
                nc.sync.dma_start(t_wac[0:1, 0:sb], d_wa[b : b + 1, 0:sb])

                # pass 2: me[e] = sum_s nodesT[e, s] * wa[s]
                if STAGE < 4:
                    continue
                for et in range(ET):
                    for ci, (c0, cw) in enumerate(chs):
                        p_w = psw.tile([128, CHUNK], F32, tag="wb")
                        nc.tensor.matmul(p_w[:, 0:cw], sones,
                                         t_wac[0:1, c0 : c0 + cw],
                                         start=True, stop=True)
                        if STAGE == 4:
                            t_junk = strm.tile([128, CHUNK], F32, tag="junk")
                            nc.vector.tensor_copy(t_junk[:, 0:cw], p_w[:, 0:cw])
                            continue
                        t_me = strm.tile([128, 1], F32, tag="me")
                        t_junk = strm.tile([128, CHUNK], F32, tag="junk")
                        nc.vector.scalar_tensor_tensor(
                            out=t_junk[:, 0:cw],
                            in0=t_nt[:, et, c0 : c0 + cw].bitcast(F32),
                            scalar=1.0,
                            in1=p_w[:, 0:cw],
                            op0=ALU.mult, op1=ALU.mult,
                            accum_out=t_me)
                        partials[(qi, et)].append(t_me)
                if STAGE >= 6 and k == K - 1:
                    for et in range(ET):
                        ps_list = partials.pop((qi, et))
                        acc = ps_list[0]
                        for i, t in enumerate(ps_list[1:]):
                            is_last = i == len(ps_list) - 2
                            if is_last:
                                dst = t_pooled[:, et, qi : qi + 1]
                            else:
                                dst = strm.tile([128, 1], F32, tag="acc")
                            nc.vector.tensor_tensor(dst, acc, t, op=ALU.add)
                            acc = dst
            # ---------------- output projection ----------------
            if STAGE < 7:
                nc.sync.dma_start(d_outT.rearrange("(t p) q -> p t q", p=128), t_gqT.bitcast(F32))
                t_plr = None
                t_outT = None
            else:
                t_plr = wts.tile([128, ET, QPC], F32R)
                nc.vector.tensor_copy(t_plr, t_pooled)
                t_outT = wts.tile([128, DT, QPC], F32)
            for mt in range(DT if STAGE >= 7 else 0):
                p_o = ps.tile([128, CHUNK], F32, tag="mm")
                for et in range(ET):
                    nc.tensor.matmul(p_o[:, 0:QPC],
                                     sWg[:, et, mt * 128:(mt + 1) * 128],
                                     t_plr[:, et, :],
                                     start=(et == 0), stop=(et == ET - 1))
                nc.scalar.activation(t_outT[:, mt, :], p_o[:, 0:QPC], AF.Tanh,
                                     bias=sbg[:, mt : mt + 1], scale=1.0)
            if STAGE >= 7:
                nc.sync.dma_start(d_outT.rearrange("(t p) q -> p t q", p=128), t_outT)

    nc.compile()
    return nc


_CACHE = {}


def kernel(**inputs) -> np.ndarray:
    per_core, S_b, segs, gnn_idx, hs = _host_prep(inputs)
    key = tuple(S_b)
    if key not in _CACHE:
        _CACHE[key] = _build_program(S_b, segs)
    nc = _CACHE[key]
    res = run_bass_kernel_spmd(nc, per_core, list(range(NCORES)))
    out = hs.copy()
    for c in range(NCORES):
        rows = res.results[c]["outT"].T      # [QPC, D]
        np.add.at(out, gnn_idx[c * QPC : (c + 1) * QPC], rows)
    return out



# revision 5
# speedup vs baseline: 13235.1921x; 13235.1921x over previous
"""Trainium2 Bass kernel for the CustomGNNLayer problem.

Strategy (data-parallel over Q, 8 queries/core on 8 cores):
  host: compute the tiny projection heads (rel softmax + prob gather, gq,
        c_q) in numpy; compact node slots per (q,k) group (drop all-zero
        padded slots; pad kept counts to PAD_MULT with a per-block class
        profile uniform across cores so one SPMD program fits all cores);
        convert nodes to bf16; fold mask / mean divisors into maskf.
  device (per core, per (q,k) block): X = Wn^T @ nodesT in bf16 (PE),
        tanh+bias on ScalarE over full-block spans, dots via DVE
        multiply-accumulate chains (gq per-partition scalar) + one PE
        ones-reduce; group softmax + global softmax on [N,M] grid; weighted
        sum of nodes via PE-broadcast wa + fused DVE multiply-reduce; final
        tanh projection -> updated rows.
  host: res = hidden_states.copy(); res[gnn_idx] += rows.
"""
import sys

sys.path.insert(0, "/opt/trn_rl_repo")

import numpy as np
import ml_dtypes

import concourse.bacc as bacc
import concourse.bass as bass
import concourse.tile as tile
from concourse import mybir
from concourse.bass_utils import run_bass_kernel_spmd

F32 = mybir.dt.float32
F32R = mybir.dt.float32r
BF16 = mybir.dt.bfloat16
AF = mybir.ActivationFunctionType
ALU = mybir.AluOpType
AX = mybir.AxisListType

Q, K, N, M = 64, 2, 32, 64
E, D, R, S = 256, 1024, 200, 8192
NCORES = 8
QPC = Q // NCORES          # 8 queries per core
NB = QPC * K               # 16 blocks per core, b = qi*K + k
PAD_MULT = 8
ET = E // 128              # 2 e-tiles
DT = D // 128              # 8 d-tiles
PSW = 512                  # psum bank width (f32)
ACTW = 3 * PSW             # activation span (3 psum banks)

BF16NP = ml_dtypes.bfloat16


def _to_bf16(a):
    """f32 -> bf16 with round-to-nearest-even, via uint bit trick (fast)."""
    u = np.ascontiguousarray(a, np.float32).view(np.uint32)
    r = ((u >> 16) & 1) + 0x7FFF
    return ((u + r) >> 16).astype(np.uint16).view(BF16NP)


def _pieces(size, step=PSW):
    return [(p0, min(step, size - p0)) for p0 in range(0, size, step)]


def _host_prep(inputs):
    hs = np.ascontiguousarray(inputs["hidden_states"], dtype=np.float32)
    nodes = np.ascontiguousarray(inputs["nodes"], dtype=np.float32)
    prob_idx = np.asarray(inputs["prob_idx"])
    gnn_idx = np.asarray(inputs["gnn_idx"]).astype(np.int64)
    rel_idx = np.asarray(inputs["rel_idx"]).astype(np.int64)
    Wc = np.asarray(inputs["Wc"], np.float32); bc = np.asarray(inputs["bc"], np.float32)
    Wq = np.asarray(inputs["Wq"], np.float32); bq = np.asarray(inputs["bq"], np.float32)
    Wn = np.asarray(inputs["Wn"], np.float32); bn = np.asarray(inputs["bn"], np.float32)
    Wg = np.asarray(inputs["Wg"], np.float32); bg = np.asarray(inputs["bg"], np.float32)

    # tiny projection heads on host
    rl = hs[rel_idx] @ Wc + bc                          # [Q,R]
    rl -= rl.max(axis=1, keepdims=True)
    np.exp(rl, out=rl)
    rel_prob = rl / rl.sum(axis=1, keepdims=True)
    probs10 = 10.0 * np.take_along_axis(
        rel_prob, prob_idx.reshape(Q, K * N), axis=1).reshape(Q, K, N)
    gq = np.tanh(hs[gnn_idx] @ Wq + bq)                 # [Q,D]
    cq = gq @ np.tanh(bn)                               # [Q]

    nz = np.any(nodes != 0.0, axis=4)          # [Q,K,N,M] kept slots
    lens = nz.sum(axis=3)                      # [Q,K,N]
    Lg = np.minimum(((np.maximum(lens, 1) + PAD_MULT - 1) // PAD_MULT) * PAD_MULT, M)

    # per-block-index profile: position-wise max of descending-sorted Lg across cores
    profiles = []   # [NB][N] descending class sizes, uniform across cores
    for qi in range(QPC):
        for k in range(K):
            seqs = [np.sort(Lg[c * QPC + qi, k])[::-1] for c in range(NCORES)]
            profiles.append(np.max(np.stack(seqs), axis=0))
    S_b = [int(p.sum()) for p in profiles]
    segs = []       # [NB] list of (L, row0, cnt, slot_off)
    for p in profiles:
        s, off = [], 0
        i = 0
        while i < N:
            j = i
            while j < N and p[j] == p[i]:
                j += 1
            L = int(p[i])
            s.append((L, i, j - i, off))
            off += L * (j - i)
            i = j
        segs.append(s)

    mask0 = (nodes[..., 0] != 0.0)             # [Q,K,N,M] reference mask

    per_core = []
    for c in range(NCORES):
        qs = np.arange(c * QPC, (c + 1) * QPC)
        nt_flat = np.empty(sum(2 * 128 * s for s in S_b), BF16NP)
        maskf = np.zeros((NB, N, M), np.float32)
        spr = np.zeros((N, NB), np.float32)
        ntoff = 0
        for qi in range(QPC):
            q = qs[qi]
            for k in range(K):
                b = qi * K + k
                prof = profiles[b]
                order = np.argsort(-Lg[q, k], kind="stable")   # ranks -> groups
                comp = np.zeros((S_b[b], E), np.float32)
                off = 0
                for rank, g in enumerate(order):
                    L = int(prof[rank])
                    keep = np.nonzero(nz[q, k, g])[0]
                    nkeep = len(keep)
                    comp[off : off + nkeep] = nodes[q, k, g, keep]
                    maskf[b, rank, :nkeep] = mask0[q, k, g, keep].astype(np.float32)
                    spr[rank, b] = probs10[q, k, g]
                    off += L
                nt = _to_bf16(comp.T)                          # [E, S_b] bf16
                sz = 2 * 128 * S_b[b]
                nt_flat[ntoff : ntoff + sz] = nt.reshape(-1)
                ntoff += sz
        maskf *= 1.0 / (N * M * K)
        # gqT [128, DT, QPC]: d = t*128 + p
        gqT = np.ascontiguousarray(
            gq[qs].reshape(QPC, DT, 128).transpose(2, 1, 0), np.float32)
        scq = np.ascontiguousarray(
            np.broadcast_to(cq[qs][None, :], (N, QPC)), np.float32)
        per_core.append({
            "nodesT": nt_flat,
            "maskf": maskf,
            "spr": spr,
            "sgq": gqT,
            "scq": scq,
        })

    shared = {
        "Wn": _to_bf16(Wn),
        "Wg": _to_bf16(Wg),
        "bn": np.ascontiguousarray(bn.reshape(DT, 128).T),
        "bg": np.ascontiguousarray(bg.reshape(DT, 128).T),
        "ones128": np.ones((1, 128), BF16NP),
    }
    for pc in per_core:
        pc.update(shared)
    return per_core, S_b, segs, gnn_idx, hs


def _build_program(S_b, segs):
    nc = bacc.Bacc("TRN2", target_bir_lowering=False, debug=False,
                   num_devices=NCORES)
    S_MAX = max(S_b)
    A_MAX = min(S_MAX, ACTW)
    NT_TOT = sum(2 * 128 * s for s in S_b)

    d_nodesT = nc.dram_tensor("nodesT", [NT_TOT], BF16, kind="ExternalInput").ap()
    d_Wn = nc.dram_tensor("Wn", [E, D], BF16, kind="ExternalInput").ap()
    d_Wg = nc.dram_tensor("Wg", [E, D], BF16, kind="ExternalInput").ap()
    d_bn = nc.dram_tensor("bn", [128, DT], F32, kind="ExternalInput").ap()
    d_bg = nc.dram_tensor("bg", [128, DT], F32, kind="ExternalInput").ap()
    d_ones = nc.dram_tensor("ones128", [1, 128], BF16, kind="ExternalInput").ap()
    d_maskf = nc.dram_tensor("maskf", [NB, N, M], F32, kind="ExternalInput").ap()
    d_spr = nc.dram_tensor("spr", [N, NB], F32, kind="ExternalInput").ap()
    d_sgq = nc.dram_tensor("sgq", [128, DT, QPC], F32, kind="ExternalInput").ap()
    d_scq = nc.dram_tensor("scq", [N, QPC], F32, kind="ExternalInput").ap()
    d_outT = nc.dram_tensor("outT", [D, QPC], F32, kind="ExternalOutput").ap()

    # DRAM scratch
    d_dots = nc.dram_tensor("sc_dots", [NB, 2048], F32).ap()
    d_wa = nc.dram_tensor("sc_wa", [NB, 2048], BF16).ap()
    d_ginv = nc.dram_tensor("sc_ginv", [NB, 1], F32).ap()

    with tile.TileContext(nc) as tc:
        with tc.tile_pool(name="wts", bufs=1) as wts, \
             tc.tile_pool(name="big", bufs=2) as big, \
             tc.tile_pool(name="zp", bufs=3) as zp, \
             tc.tile_pool(name="accp", bufs=4) as accp, \
             tc.tile_pool(name="sml", bufs=4) as sml, \
             tc.tile_pool(name="mep", bufs=24) as mep, \
             tc.tile_pool(name="jnk", bufs=2) as jnk, \
             tc.tile_pool(name="ps", bufs=2, space="PSUM") as ps, \
             tc.tile_pool(name="psd", bufs=1, space="PSUM") as psd, \
             tc.tile_pool(name="psw", bufs=1, space="PSUM") as psw:

            # ---------------- load constants ----------------
            sWn = wts.tile([128, ET, D], BF16)
            nc.sync.dma_start(sWn, d_Wn.rearrange("(t p) r -> p t r", p=128))
            sWg = wts.tile([128, ET, D], BF16)
            nc.sync.dma_start(sWg, d_Wg.rearrange("(t p) r -> p t r", p=128))
            sbn = wts.tile([128, DT], F32)
            nc.sync.dma_start(sbn, d_bn)
            sbg = wts.tile([128, DT], F32)
            nc.sync.dma_start(sbg, d_bg)
            sones_r = wts.tile([1, 128], BF16)
            nc.sync.dma_start(sones_r, d_ones)
            sones_c = wts.tile([128, 1], BF16)
            nc.sync.dma_start(sones_c, d_ones.rearrange("o p -> p o"))
            smaskf = wts.tile([N, NB, M], F32)
            nc.sync.dma_start(smaskf, d_maskf.rearrange("b n m -> n b m"))
            spr = wts.tile([N, NB], F32)
            nc.sync.dma_start(spr, d_spr)
            sgq = wts.tile([128, DT, QPC], F32)
            nc.sync.dma_start(sgq, d_sgq)
            scq = wts.tile([N, QPC], F32)
            nc.sync.dma_start(scq, d_scq)

            # ---------------- main loop ----------------
            t_pooled = wts.tile([128, ET, QPC], F32)
            nt_off = 0
            from collections import defaultdict
            partials = defaultdict(list)
            for b in range(NB):
                qi, k = b // K, b % K
                sb = S_b[b]

                t_nt = big.tile([128, ET, S_MAX], BF16, tag="nt")
                nc.sync.dma_start(
                    t_nt[:, :, 0:sb],
                    bass.AP(tensor=d_nodesT.tensor, offset=nt_off,
                            ap=[[sb, 128], [128 * sb, ET], [1, sb]]))
                nt_off += 2 * 128 * sb

                # ---- X = Wn^T @ nt, tanh, dots accumulation ----
                t_dots = sml.tile([1, S_MAX], F32, tag="dots")
                for a0 in range(0, sb, ACTW):
                    aw = min(ACTW, sb - a0)
                    acc = [None, None]   # parity chains
                    for dt_i in range(DT):
                        p_x = ps.tile([128, ACTW], F32, tag="mm")
                        for (p0, pw) in _pieces(aw):
                            for et in range(ET):
                                nc.tensor.matmul(
                                    p_x[:, p0 : p0 + pw],
                                    sWn[:, et, dt_i * 128:(dt_i + 1) * 128],
                                    t_nt[:, et, a0 + p0 : a0 + p0 + pw],
                                    start=(et == 0), stop=(et == ET - 1))
                        t_z = zp.tile([128, ACTW], BF16, tag="z")
                        nc.scalar.activation(t_z[:, 0:aw], p_x[:, 0:aw],
                                             AF.Tanh, bias=sbn[:, dt_i : dt_i + 1],
                                             scale=1.0)
                        par = dt_i & 1
                        gqc = sgq[:, dt_i, qi : qi + 1]
                        if acc[par] is None:
                            a_t = accp.tile([128, ACTW], BF16, tag="acc")
                            nc.vector.tensor_scalar_mul(a_t[:, 0:aw], t_z[:, 0:aw], gqc)
                            acc[par] = a_t
                        else:
                            a_t = accp.tile([128, ACTW], BF16, tag="acc")
                            nc.vector.scalar_tensor_tensor(
                                a_t[:, 0:aw], t_z[:, 0:aw], gqc, acc[par][:, 0:aw],
                                op0=ALU.mult, op1=ALU.add)
                            acc[par] = a_t
                    # dots reduce: ones^T @ (acc0 + acc1), via 2 accumulating matmuls
                    for (p0, pw) in _pieces(aw):
                        p_d = psd.tile([1, PSW], F32, tag="dr")
                        nc.tensor.matmul(p_d[0:1, 0:pw], sones_c,
                                         acc[0][:, p0 : p0 + pw],
                                         start=True, stop=False)
                        nc.tensor.matmul(p_d[0:1, 0:pw], sones_c,
                                         acc[1][:, p0 : p0 + pw],
                                         start=False, stop=True)
                        nc.vector.tensor_copy(t_dots[0:1, a0 + p0 : a0 + p0 + pw],
                                              p_d[0:1, 0:pw])
                nc.sync.dma_start(d_dots[b : b + 1, 0:sb], t_dots[0:1, 0:sb])

                # ---- softmax on [N, M] grid ----
                t_dbuf = sml.tile([N, M], F32, tag="dbuf")
                nc.vector.tensor_scalar(t_dbuf, smaskf[:, b, :], 0.0,
                                        scq[:, qi : qi + 1],
                                        op0=ALU.mult, op1=ALU.add)
                for (L, r0, cnt, soff) in segs[b]:
                    nc.sync.dma_start(
                        t_dbuf[r0 : r0 + cnt, 0:L],
                        d_dots[b, soff : soff + cnt * L].rearrange("(c l) -> c l", l=L))

                t_gmx = sml.tile([N, 1], F32, tag="gmx")
                nc.vector.tensor_reduce(t_gmx, t_dbuf, axis=AX.X, op=ALU.max)
                t_gnmx = sml.tile([N, 1], F32, tag="gnmx")
                nc.vector.tensor_scalar_mul(t_gnmx, t_gmx, -1.0)
                t_ex = sml.tile([N, M], F32, tag="ex")
                t_rs = sml.tile([N, 1], F32, tag="rs")
                nc.scalar.activation(t_ex, t_dbuf, AF.Exp, bias=t_gnmx, scale=1.0,
                                     accum_out=t_rs)
                t_ri = sml.tile([N, 1], F32, tag="ri")
                nc.vector.reciprocal(t_ri, t_rs)
                t_lg = sml.tile([N, M], F32, tag="lg")
                nc.vector.tensor_scalar(t_lg, t_ex, t_ri, spr[:, b : b + 1],
                                        op0=ALU.mult, op1=ALU.mult)
                t_gl = sml.tile([N, M], F32, tag="gl")
                t_grs = sml.tile([N, 1], F32, tag="grs")
                nc.scalar.activation(t_gl, t_lg, AF.Exp, accum_out=t_grs)
                t_gs = sml.tile([1, 1], F32, tag="gs")
                nc.gpsimd.tensor_reduce(t_gs, t_grs, axis=AX.C, op=ALU.add)
                t_gi = sml.tile([1, 1], F32, tag="gi")
                nc.vector.reciprocal(t_gi, t_gs)
                nc.sync.dma_start(d_ginv[b : b + 1, :], t_gi)
                t_gic = sml.tile([N, 1], F32, tag="gic")
                nc.sync.dma_start(
                    t_gic,
                    bass.AP(tensor=d_ginv.tensor, offset=b, ap=[[0, N], [1, 1]]))
                t_wa = sml.tile([N, M], BF16, tag="wa")
                nc.vector.scalar_tensor_tensor(
                    t_wa, t_gl, t_gic, smaskf[:, b, :],
                    op0=ALU.mult, op1=ALU.mult)

                # gather back to compacted order
                for (L, r0, cnt, soff) in segs[b]:
                    nc.sync.dma_start(
                        d_wa[b, soff : soff + cnt * L].rearrange("(c l) -> c l", l=L),
                        t_wa[r0 : r0 + cnt, 0:L])
                t_wac = sml.tile([1, S_MAX], BF16, tag="wac")
                nc.sync.dma_start(t_wac[0:1, 0:sb], d_wa[b : b + 1, 0:sb])

                # ---- pass 2: me[e] = sum_s nodesT[e, s] * wa[s] ----
                for (p0, pw) in _pieces(sb):
                    p_w = psw.tile([128, PSW], F32, tag="wb")
                    nc.tensor.matmul(p_w[:, 0:pw], sones_r,
                                     t_wac[0:1, p0 : p0 + pw],
                                     start=True, stop=True)
                    for et in range(ET):
                        t_me = mep.tile([128, 1], F32, tag="me")
                        t_junk = jnk.tile([128, PSW], BF16, tag="junk")
                        nc.vector.scalar_tensor_tensor(
                            out=t_junk[:, 0:pw],
                            in0=t_nt[:, et, p0 : p0 + pw],
                            scalar=1.0,
                            in1=p_w[:, 0:pw],
                            op0=ALU.mult, op1=ALU.mult,
                            accum_out=t_me)
                        partials[(qi, et)].append(t_me)
                if k == K - 1:
                    for et in range(ET):
                        ps_list = partials.pop((qi, et))
                        acc_t = ps_list[0]
                        for i, t in enumerate(ps_list[1:]):
                            is_last = i == len(ps_list) - 2
                            if is_last:
                                dst = t_pooled[:, et, qi : qi + 1]
                            else:
                                dst = mep.tile([128, 1], F32, tag="macc")
                            nc.vector.tensor_tensor(dst, acc_t, t, op=ALU.add)
                            acc_t = dst

            # ---------------- output projection ----------------
            t_plr = wts.tile([128, ET, QPC], BF16)
            nc.vector.tensor_copy(t_plr, t_pooled)
            t_outT = wts.tile([128, DT, QPC], F32)
            for mt in range(DT):
                p_o = ps.tile([128, ACTW], F32, tag="mm")
                for et in range(ET):
                    nc.tensor.matmul(p_o[:, 0:QPC],
                                     sWg[:, et, mt * 128:(mt + 1) * 128],
                                     t_plr[:, et, :],
                                     start=(et == 0), stop=(et == ET - 1))
                nc.scalar.activation(t_outT[:, mt, :], p_o[:, 0:QPC], AF.Tanh,
                                     bias=sbg[:, mt : mt + 1], scale=1.0)
            nc.sync.dma_start(d_outT.rearrange("(t p) q -> p t q", p=128), t_outT)

    nc.compile()
    return nc


_CACHE = {}


def kernel(**inputs) -> np.ndarray:
    per_core, S_b, segs, gnn_idx, hs = _host_prep(inputs)
    key = tuple(S_b)
    if key not in _CACHE:
        _CACHE[key] = _build_program(S_b, segs)
    nc = _CACHE[key]
    res = run_bass_kernel_spmd(nc, per_core, list(range(NCORES)))
    out = hs.copy()
    for c in range(NCORES):
        rows = res.results[c]["outT"].T      # [QPC, D]
        np.add.at(out, gnn_idx[c * QPC : (c + 1) * QPC], rows)
    return out


# revision 20
# speedup vs baseline: 19764.0879x; 1.4933x over previous
"""Trainium2 Bass kernel for the CustomGNNLayer problem.

Strategy (data-parallel over Q, 8 queries/core on 8 cores):
  host: compute the tiny projection heads (rel softmax + prob gather, gq,
        c_q) in numpy; compact node slots per (q,k) group (drop all-zero
        padded slots; pad kept counts to PAD_MULT with a per-block class
        profile uniform across cores so one SPMD program fits all cores);
        convert nodes to bf16; fold mask / mean divisors into maskf.
  device (per core, per (q,k) block): X = Wn^T @ nodesT in bf16 (PE),
        tanh+bias on ScalarE over full-block spans, dots via DVE
        multiply-accumulate chains (gq per-partition scalar) + one PE
        ones-reduce; group softmax + global softmax on [N,M] grid; weighted
        sum of nodes via PE-broadcast wa + fused DVE multiply-reduce; final
        tanh projection -> updated rows.
  host: res = hidden_states.copy(); res[gnn_idx] += rows.
"""
import sys

sys.path.insert(0, "/opt/trn_rl_repo")

import numpy as np
import ml_dtypes

import concourse.bacc as bacc
import concourse.bass as bass
import concourse.tile as tile
from concourse import mybir
from concourse.bass_utils import run_bass_kernel_spmd

F32 = mybir.dt.float32
F32R = mybir.dt.float32r
BF16 = mybir.dt.bfloat16
AF = mybir.ActivationFunctionType
ALU = mybir.AluOpType
AX = mybir.AxisListType

Q, K, N, M = 64, 2, 32, 64
E, D, R, S = 256, 1024, 200, 8192
NCORES = 8
QPC = Q // NCORES          # 8 queries per core
NB = QPC * K               # 16 blocks per core, b = qi*K + k
PAD_MULT = 8
ET = E // 128              # 2 e-tiles
DT = D // 128              # 8 d-tiles
PSW = 512                  # psum bank width (f32)
ACTW = 3 * PSW             # activation span (3 psum banks)

BF16NP = ml_dtypes.bfloat16


def _to_bf16(a):
    """f32 -> bf16 with round-to-nearest-even, via uint bit trick (fast)."""
    u = np.ascontiguousarray(a, np.float32).view(np.uint32)
    r = ((u >> 16) & 1) + 0x7FFF
    return ((u + r) >> 16).astype(np.uint16).view(BF16NP)


def _pieces(size, step=PSW):
    return [(p0, min(step, size - p0)) for p0 in range(0, size, step)]


def _host_prep(inputs):
    hs = np.ascontiguousarray(inputs["hidden_states"], dtype=np.float32)
    nodes = np.ascontiguousarray(inputs["nodes"], dtype=np.float32)
    prob_idx = np.asarray(inputs["prob_idx"])
    gnn_idx = np.asarray(inputs["gnn_idx"]).astype(np.int64)
    rel_idx = np.asarray(inputs["rel_idx"]).astype(np.int64)
    Wc = np.asarray(inputs["Wc"], np.float32); bc = np.asarray(inputs["bc"], np.float32)
    Wq = np.asarray(inputs["Wq"], np.float32); bq = np.asarray(inputs["bq"], np.float32)
    Wn = np.asarray(inputs["Wn"], np.float32); bn = np.asarray(inputs["bn"], np.float32)
    Wg = np.asarray(inputs["Wg"], np.float32); bg = np.asarray(inputs["bg"], np.float32)

    # tiny projection heads on host
    rl = hs[rel_idx] @ Wc + bc                          # [Q,R]
    rl -= rl.max(axis=1, keepdims=True)
    np.exp(rl, out=rl)
    rel_prob = rl / rl.sum(axis=1, keepdims=True)
    probs10 = 10.0 * np.take_along_axis(
        rel_prob, prob_idx.reshape(Q, K * N), axis=1).reshape(Q, K, N)
    gq = np.tanh(hs[gnn_idx] @ Wq + bq)                 # [Q,D]
    cq = gq @ np.tanh(bn)                               # [Q]

    nz = np.any(nodes != 0.0, axis=4)          # [Q,K,N,M] kept slots
    lens = nz.sum(axis=3)                      # [Q,K,N]
    Lg = np.minimum(((np.maximum(lens, 1) + PAD_MULT - 1) // PAD_MULT) * PAD_MULT, M)

    # per-block-index profile: position-wise max of descending-sorted Lg across cores
    profiles = []   # [NB][N] descending class sizes, uniform across cores
    for qi in range(QPC):
        for k in range(K):
            seqs = [np.sort(Lg[c * QPC + qi, k])[::-1] for c in range(NCORES)]
            profiles.append(np.max(np.stack(seqs), axis=0))
    S_b = [int(p.sum()) for p in profiles]
    segs = []       # [NB] list of (L, row0, cnt, slot_off)
    for p in profiles:
        s, off = [], 0
        i = 0
        while i < N:
            j = i
            while j < N and p[j] == p[i]:
                j += 1
            L = int(p[i])
            s.append((L, i, j - i, off))
            off += L * (j - i)
            i = j
        segs.append(s)

    mask0 = (nodes[..., 0] != 0.0)             # [Q,K,N,M] reference mask

    per_core = []
    for c in range(NCORES):
        qs = np.arange(c * QPC, (c + 1) * QPC)
        nt_flat = np.empty(sum(2 * 128 * s for s in S_b), BF16NP)
        maskf = np.zeros((NB, N, M), np.float32)
        spr = np.zeros((N, NB), np.float32)
        ntoff = 0
        for qi in range(QPC):
            q = qs[qi]
            for k in range(K):
                b = qi * K + k
                prof = profiles[b]
                order = np.argsort(-Lg[q, k], kind="stable")   # ranks -> groups
                comp = np.zeros((S_b[b], E), np.float32)
                off = 0
                for rank, g in enumerate(order):
                    L = int(prof[rank])
                    keep = np.nonzero(nz[q, k, g])[0]
                    nkeep = len(keep)
                    comp[off : off + nkeep] = nodes[q, k, g, keep]
                    maskf[b, rank, :nkeep] = mask0[q, k, g, keep].astype(np.float32)
                    spr[rank, b] = probs10[q, k, g]
                    off += L
                nt = _to_bf16(comp.T)                          # [E, S_b] bf16
                sz = 2 * 128 * S_b[b]
                nt_flat[ntoff : ntoff + sz] = nt.reshape(-1)
                ntoff += sz
        maskf *= 1.0 / (N * M * K)
        # gqT [128, DT, QPC]: d = t*128 + p
        gqT = np.ascontiguousarray(
            gq[qs].reshape(QPC, DT, 128).transpose(2, 1, 0), np.float32)
        scq = np.ascontiguousarray(
            np.broadcast_to(cq[qs][None, :], (N, QPC)), np.float32)
        per_core.append({
            "nodesT": nt_flat,
            "maskf": maskf,
            "spr": spr,
            "sgq": gqT,
            "scq": scq,
        })

    shared = {
        "Wn": _to_bf16(Wn),
        "Wg": _to_bf16(Wg),
        "bn": np.ascontiguousarray(bn.reshape(DT, 128).T),
        "bg": np.ascontiguousarray(bg.reshape(DT, 128).T),
        "ones128": np.ones((1, 128), BF16NP),
    }
    for pc in per_core:
        pc.update(shared)
    return per_core, S_b, segs, gnn_idx, hs


def _build_program(S_b, segs):
    nc = bacc.Bacc("TRN2", target_bir_lowering=False, debug=False,
                   num_devices=NCORES)
    S_MAX = max(S_b)
    A_MAX = min(S_MAX, ACTW)
    NT_TOT = sum(2 * 128 * s for s in S_b)

    d_nodesT = nc.dram_tensor("nodesT", [NT_TOT], BF16, kind="ExternalInput").ap()
    d_Wn = nc.dram_tensor("Wn", [E, D], BF16, kind="ExternalInput").ap()
    d_Wg = nc.dram_tensor("Wg", [E, D], BF16, kind="ExternalInput").ap()
    d_bn = nc.dram_tensor("bn", [128, DT], F32, kind="ExternalInput").ap()
    d_bg = nc.dram_tensor("bg", [128, DT], F32, kind="ExternalInput").ap()
    d_ones = nc.dram_tensor("ones128", [1, 128], BF16, kind="ExternalInput").ap()
    d_maskf = nc.dram_tensor("maskf", [NB, N, M], F32, kind="ExternalInput").ap()
    d_spr = nc.dram_tensor("spr", [N, NB], F32, kind="ExternalInput").ap()
    d_sgq = nc.dram_tensor("sgq", [128, DT, QPC], F32, kind="ExternalInput").ap()
    d_scq = nc.dram_tensor("scq", [N, QPC], F32, kind="ExternalInput").ap()
    d_outT = nc.dram_tensor("outT", [D, QPC], F32, kind="ExternalOutput").ap()

    # DRAM scratch
    d_dots = nc.dram_tensor("sc_dots", [NB, 2048], F32).ap()
    d_wa = nc.dram_tensor("sc_wa", [NB, 2048], BF16).ap()
    d_ginv = nc.dram_tensor("sc_ginv", [NB, 1], F32).ap()

    with tile.TileContext(nc) as tc:
        with tc.tile_pool(name="wts", bufs=1) as wts, \
             tc.tile_pool(name="big", bufs=4) as big, \
             tc.tile_pool(name="zp", bufs=3) as zp, \
             tc.tile_pool(name="zzp", bufs=4) as zzp, \
             tc.tile_pool(name="accp", bufs=6) as accp, \
             tc.tile_pool(name="sml", bufs=6) as sml, \
             tc.tile_pool(name="mep", bufs=24) as mep, \
             tc.tile_pool(name="jnk", bufs=2) as jnk, \
             tc.tile_pool(name="ps", bufs=2, space="PSUM") as ps, \
             tc.tile_pool(name="psd", bufs=1, space="PSUM") as psd, \
             tc.tile_pool(name="psw", bufs=1, space="PSUM") as psw:

            # ---------------- load constants ----------------
            sWn = wts.tile([128, ET, D], BF16)
            nc.sync.dma_start(sWn, d_Wn.rearrange("(t p) r -> p t r", p=128))
            sWg = wts.tile([128, ET, D], BF16)
            nc.sync.dma_start(sWg, d_Wg.rearrange("(t p) r -> p t r", p=128))
            sbn = wts.tile([128, DT], F32)
            nc.sync.dma_start(sbn, d_bn)
            sbg = wts.tile([128, DT], F32)
            nc.sync.dma_start(sbg, d_bg)
            sones_r = wts.tile([1, 128], BF16)
            nc.sync.dma_start(sones_r, d_ones)
            sones_c = wts.tile([128, 1], BF16)
            nc.sync.dma_start(sones_c, d_ones.rearrange("o p -> p o"))
            smaskf = wts.tile([N, NB, M], F32)
            nc.sync.dma_start(smaskf, d_maskf.rearrange("b n m -> n b m"))
            spr = wts.tile([N, NB], F32)
            nc.sync.dma_start(spr, d_spr)
            sgq = wts.tile([128, DT, QPC], F32)
            nc.sync.dma_start(sgq, d_sgq)
            scq = wts.tile([N, QPC], F32)
            nc.sync.dma_start(scq, d_scq)

            # ---------------- main loop ----------------
            t_pooled = wts.tile([128, ET, QPC], F32)
            nt_off = 0
            from collections import defaultdict
            partials = defaultdict(list)
            for b in range(NB):
                qi, k = b // K, b % K
                sb = S_b[b]

                t_nt = big.tile([128, ET, S_MAX], BF16, tag="nt")
                nc.sync.dma_start(
                    t_nt[:, :, 0:sb],
                    bass.AP(tensor=d_nodesT.tensor, offset=nt_off,
                            ap=[[sb, 128], [128 * sb, ET], [1, sb]]))
                nt_off += 2 * 128 * sb

                # ---- X = Wn^T @ nt, tanh, dots accumulation ----
                t_dots = sml.tile([1, S_MAX], F32, tag="dots")
                for a0 in range(0, sb, ACTW):
                    aw = min(ACTW, sb - a0)
                    acc = [None, None]   # parity chains
                    for dt_i in range(DT):
                        p_x = ps.tile([128, ACTW], F32, tag="mm")
                        for (p0, pw) in _pieces(aw):
                            for et in range(ET):
                                nc.tensor.matmul(
                                    p_x[:, p0 : p0 + pw],
                                    sWn[:, et, dt_i * 128:(dt_i + 1) * 128],
                                    t_nt[:, et, a0 + p0 : a0 + p0 + pw],
                                    start=(et == 0), stop=(et == ET - 1))
                        t_z = zp.tile([128, ACTW], BF16, tag="z")
                        nc.scalar.activation(t_z[:, 0:aw], p_x[:, 0:aw],
                                             AF.Tanh, bias=sbn[:, dt_i : dt_i + 1],
                                             scale=1.0)
                        par = dt_i & 1
                        gqc = sgq[:, dt_i, qi : qi + 1]
                        # 4x-mode multiply, then 2x-mode bf16 add chain
                        t_zz = zzp.tile([128, ACTW], BF16, tag="zz")
                        nc.vector.tensor_scalar_mul(t_zz[:, 0:aw], t_z[:, 0:aw], gqc)
                        if acc[par] is None:
                            acc[par] = t_zz
                        else:
                            a_t = accp.tile([128, ACTW], BF16, tag="acc")
                            nc.vector.tensor_tensor(
                                a_t[:, 0:aw], t_zz[:, 0:aw], acc[par][:, 0:aw],
                                op=ALU.add)
                            acc[par] = a_t
                    # dots reduce: ones^T @ (acc0 + acc1), via 2 accumulating matmuls
                    for (p0, pw) in _pieces(aw):
                        p_d = psd.tile([1, PSW], F32, tag="dr")
                        nc.tensor.matmul(p_d[0:1, 0:pw], sones_c,
                                         acc[0][:, p0 : p0 + pw],
                                         start=True, stop=False)
                        nc.tensor.matmul(p_d[0:1, 0:pw], sones_c,
                                         acc[1][:, p0 : p0 + pw],
                                         start=False, stop=True)
                        nc.vector.tensor_copy(t_dots[0:1, a0 + p0 : a0 + p0 + pw],
                                              p_d[0:1, 0:pw])
                nc.sync.dma_start(d_dots[b : b + 1, 0:sb], t_dots[0:1, 0:sb])

                # ---- softmax on [N, M] grid ----
                t_dbuf = sml.tile([N, M], F32, tag="dbuf")
                nc.gpsimd.tensor_scalar(t_dbuf, smaskf[:, b, :], 0.0,
                                        scq[:, qi : qi + 1],
                                        op0=ALU.mult, op1=ALU.add)
                for (L, r0, cnt, soff) in segs[b]:
                    nc.sync.dma_start(
                        t_dbuf[r0 : r0 + cnt, 0:L],
                        d_dots[b, soff : soff + cnt * L].rearrange("(c l) -> c l", l=L))

                t_gmx = sml.tile([N, 1], F32, tag="gmx")
                nc.vector.tensor_reduce(t_gmx, t_dbuf, axis=AX.X, op=ALU.max)
                t_gnmx = sml.tile([N, 1], F32, tag="gnmx")
                nc.gpsimd.tensor_scalar_mul(t_gnmx, t_gmx, -1.0)
                t_ex = sml.tile([N, M], F32, tag="ex")
                t_rs = sml.tile([N, 1], F32, tag="rs")
                nc.scalar.activation(t_ex, t_dbuf, AF.Exp, bias=t_gnmx, scale=1.0,
                                     accum_out=t_rs)
                t_ri = sml.tile([N, 1], F32, tag="ri")
                nc.vector.reciprocal(t_ri, t_rs)
                t_lg = sml.tile([N, M], F32, tag="lg")
                nc.vector.tensor_scalar(t_lg, t_ex, t_ri, spr[:, b : b + 1],
                                        op0=ALU.mult, op1=ALU.mult)
                t_gl = sml.tile([N, M], F32, tag="gl")
                t_grs = sml.tile([N, 1], F32, tag="grs")
                nc.scalar.activation(t_gl, t_lg, AF.Exp, accum_out=t_grs)
                t_gs = sml.tile([1, 1], F32, tag="gs")
                nc.gpsimd.tensor_reduce(t_gs, t_grs, axis=AX.C, op=ALU.add)
                t_gi = sml.tile([1, 1], F32, tag="gi")
                nc.vector.reciprocal(t_gi, t_gs)
                nc.sync.dma_start(d_ginv[b : b + 1, :], t_gi)
                t_gic = sml.tile([N, 1], F32, tag="gic")
                nc.sync.dma_start(
                    t_gic,
                    bass.AP(tensor=d_ginv.tensor, offset=b, ap=[[0, N], [1, 1]]))
                t_wa = sml.tile([N, M], BF16, tag="wa")
                nc.vector.scalar_tensor_tensor(
                    t_wa, t_gl, t_gic, smaskf[:, b, :],
                    op0=ALU.mult, op1=ALU.mult)

                # gather back to compacted order
                for (L, r0, cnt, soff) in segs[b]:
                    nc.sync.dma_start(
                        d_wa[b, soff : soff + cnt * L].rearrange("(c l) -> c l", l=L),
                        t_wa[r0 : r0 + cnt, 0:L])
                t_wac = sml.tile([1, S_MAX], BF16, tag="wac")
                nc.sync.dma_start(t_wac[0:1, 0:sb], d_wa[b : b + 1, 0:sb])

                # ---- pass 2: me[e] = sum_s nodesT[e, s] * wa[s] ----
                for (p0, pw) in _pieces(sb):
                    p_w = psw.tile([128, PSW], F32, tag="wb")
                    nc.tensor.matmul(p_w[:, 0:pw], sones_r,
                                     t_wac[0:1, p0 : p0 + pw],
                                     start=True, stop=True)
                    for et in range(ET):
                        t_me = mep.tile([128, 1], F32, tag="me")
                        t_junk = jnk.tile([128, PSW], BF16, tag="junk")
                        nc.vector.scalar_tensor_tensor(
                            out=t_junk[:, 0:pw],
                            in0=t_nt[:, et, p0 : p0 + pw],
                            scalar=1.0,
                            in1=p_w[:, 0:pw],
                            op0=ALU.mult, op1=ALU.mult,
                            accum_out=t_me)
                        partials[(qi, et)].append(t_me)
                if k == K - 1:
                    for et in range(ET):
                        ps_list = partials.pop((qi, et))
                        acc_t = ps_list[0]
                        for i, t in enumerate(ps_list[1:]):
                            is_last = i == len(ps_list) - 2
                            if is_last:
                                dst = t_pooled[:, et, qi : qi + 1]
                            else:
                                dst = mep.tile([128, 1], F32, tag="macc")
                            nc.vector.tensor_tensor(dst, acc_t, t, op=ALU.add)
                            acc_t = dst

            # ---------------- output projection ----------------
            t_plr = wts.tile([128, ET, QPC], BF16)
            nc.vector.tensor_copy(t_plr, t_pooled)
            t_outT = wts.tile([128, DT, QPC], F32)
            for mt in range(DT):
                p_o = ps.tile([128, ACTW], F32, tag="mm")
                for et in range(ET):
                    nc.tensor.matmul(p_o[:, 0:QPC],
                                     sWg[:, et, mt * 128:(mt + 1) * 128],
                                     t_plr[:, et, :],
                                     start=(et == 0), stop=(et == ET - 1))
                nc.scalar.activation(t_outT[:, mt, :], p_o[:, 0:QPC], AF.Tanh,
                                     bias=sbg[:, mt : mt + 1], scale=1.0)
            nc.sync.dma_start(d_outT.rearrange("(t p) q -> p t q", p=128), t_outT)

    nc.compile()
    return nc


_CACHE = {}


def kernel(**inputs) -> np.ndarray:
    per_core, S_b, segs, gnn_idx, hs = _host_prep(inputs)
    key = tuple(S_b)
    if key not in _CACHE:
        _CACHE[key] = _build_program(S_b, segs)
    nc = _CACHE[key]
    res = run_bass_kernel_spmd(nc, per_core, list(range(NCORES)))
    out = hs.copy()
    for c in range(NCORES):
        rows = res.results[c]["outT"].T      # [QPC, D]
        np.add.at(out, gnn_idx[c * QPC : (c + 1) * QPC], rows)
    return out


# revision 21
# speedup vs baseline: 21694.9684x; 1.0977x over previous
"""Trainium2 Bass kernel for the CustomGNNLayer problem.

Strategy (data-parallel over Q, 8 queries/core on 8 cores):
  host: compute the tiny projection heads (rel softmax + prob gather, gq,
        c_q) in numpy; compact node slots per (q,k) group (drop all-zero
        padded slots; pad kept counts to PAD_MULT with a per-block class
        profile uniform across cores so one SPMD program fits all cores,
        then pad each block to a 16-slot multiple); nodes+Wn in fp8e4,
        everything big else bf16; fold mask / mean divisors into maskf.
  device (per core, per (q,k) block): X = Wn^T @ nodesT in one fp8
        DoubleRow matmul per (d-tile, piece); tanh+bias on ScalarE; dots
        via per-d-tile PE matmuls with gq as 1-column stationary weights,
        accumulated in PSUM; group softmax + global softmax on [N,M] grid;
        weighted sum of nodes via PE-broadcast wa + fused DVE
        multiply-reduce; final tanh projection -> updated rows.
  host: res = hidden_states.copy(); res[gnn_idx] += rows.
"""
import sys

sys.path.insert(0, "/opt/trn_rl_repo")

import numpy as np
import ml_dtypes

import concourse.bacc as bacc
import concourse.bass as bass
import concourse.tile as tile
from concourse import mybir
from concourse.bass_utils import run_bass_kernel_spmd

F32 = mybir.dt.float32
BF16 = mybir.dt.bfloat16
FP8 = mybir.dt.float8e4
AF = mybir.ActivationFunctionType
ALU = mybir.AluOpType
AX = mybir.AxisListType
DR = mybir.MatmulPerfMode.DoubleRow

Q, K, N, M = 64, 2, 32, 64
E, D, R, S = 256, 1024, 200, 8192
NCORES = 8
QPC = Q // NCORES          # 8 queries per core
NB = QPC * K               # 16 blocks per core, b = qi*K + k
PAD_MULT = 8
ET = E // 128              # 2 e-tiles
DT = D // 128              # 8 d-tiles
PSW = 512                  # psum bank width (f32)
ACTW = 3 * PSW             # activation span (3 psum banks)

BF16NP = ml_dtypes.bfloat16
FP8NP = ml_dtypes.float8_e4m3fn


def _pieces(size, step=PSW):
    return [(p0, min(step, size - p0)) for p0 in range(0, size, step)]


def _host_prep(inputs):
    hs = np.ascontiguousarray(inputs["hidden_states"], dtype=np.float32)
    nodes = np.ascontiguousarray(inputs["nodes"], dtype=np.float32)
    prob_idx = np.asarray(inputs["prob_idx"])
    gnn_idx = np.asarray(inputs["gnn_idx"]).astype(np.int64)
    rel_idx = np.asarray(inputs["rel_idx"]).astype(np.int64)
    Wc = np.asarray(inputs["Wc"], np.float32); bc = np.asarray(inputs["bc"], np.float32)
    Wq = np.asarray(inputs["Wq"], np.float32); bq = np.asarray(inputs["bq"], np.float32)
    Wn = np.asarray(inputs["Wn"], np.float32); bn = np.asarray(inputs["bn"], np.float32)
    Wg = np.asarray(inputs["Wg"], np.float32); bg = np.asarray(inputs["bg"], np.float32)

    # tiny projection heads on host
    rl = hs[rel_idx] @ Wc + bc                          # [Q,R]
    rl -= rl.max(axis=1, keepdims=True)
    np.exp(rl, out=rl)
    rel_prob = rl / rl.sum(axis=1, keepdims=True)
    probs10 = 10.0 * np.take_along_axis(
        rel_prob, prob_idx.reshape(Q, K * N), axis=1).reshape(Q, K, N)
    gq = np.tanh(hs[gnn_idx] @ Wq + bq)                 # [Q,D]
    cq = gq @ np.tanh(bn)                               # [Q]

    nz = np.any(nodes != 0.0, axis=4)          # [Q,K,N,M] kept slots
    lens = nz.sum(axis=3)                      # [Q,K,N]
    Lg = np.minimum(((np.maximum(lens, 1) + PAD_MULT - 1) // PAD_MULT) * PAD_MULT, M)

    # per-block-index profile: position-wise max of descending-sorted Lg across cores
    profiles = []   # [NB][N] descending class sizes, uniform across cores
    for qi in range(QPC):
        for k in range(K):
            seqs = [np.sort(Lg[c * QPC + qi, k])[::-1] for c in range(NCORES)]
            profiles.append(np.max(np.stack(seqs), axis=0))
    S_raw = [int(p.sum()) for p in profiles]
    S_b = [((s + 15) // 16) * 16 for s in S_raw]   # pad to 16 for fp8 APs
    segs = []       # [NB] list of (L, row0, cnt, slot_off)
    for p in profiles:
        s, off = [], 0
        i = 0
        while i < N:
            j = i
            while j < N and p[j] == p[i]:
                j += 1
            L = int(p[i])
            s.append((L, i, j - i, off))
            off += L * (j - i)
            i = j
        segs.append(s)

    mask0 = (nodes[..., 0] != 0.0)             # [Q,K,N,M] reference mask

    per_core = []
    for c in range(NCORES):
        qs = np.arange(c * QPC, (c + 1) * QPC)
        nt_flat = np.empty(sum(2 * 128 * s for s in S_b), FP8NP)
        maskf = np.zeros((NB, N, M), np.float32)
        spr = np.zeros((N, NB), np.float32)
        ntoff = 0
        for qi in range(QPC):
            q = qs[qi]
            for k in range(K):
                b = qi * K + k
                prof = profiles[b]
                order = np.argsort(-Lg[q, k], kind="stable")   # ranks -> groups
                comp = np.zeros((S_b[b], E), np.float32)
                off = 0
                for rank, g in enumerate(order):
                    L = int(prof[rank])
                    keep = np.nonzero(nz[q, k, g])[0]
                    nkeep = len(keep)
                    comp[off : off + nkeep] = nodes[q, k, g, keep]
                    maskf[b, rank, :nkeep] = mask0[q, k, g, keep].astype(np.float32)
                    spr[rank, b] = probs10[q, k, g]
                    off += L
                nt = comp.T.astype(FP8NP)                      # [E, S_b] fp8
                sz = 2 * 128 * S_b[b]
                nt_flat[ntoff : ntoff + sz] = nt.reshape(-1)
                ntoff += sz
        maskf *= 1.0 / (N * M * K)
        # gqT [128, DT, QPC]: d = t*128 + p
        gqT = np.ascontiguousarray(
            gq[qs].reshape(QPC, DT, 128).transpose(2, 1, 0)).astype(BF16NP)
        scq = np.ascontiguousarray(
            np.broadcast_to(cq[qs][None, :], (N, QPC)), np.float32)
        per_core.append({
            "nodesT": nt_flat,
            "maskf": maskf,
            "spr": spr,
            "sgq": gqT,
            "scq": scq,
        })

    shared = {
        "Wn": Wn.astype(FP8NP),
        "Wg": Wg.astype(BF16NP),
        "bn": np.ascontiguousarray(bn.reshape(DT, 128).T),
        "bg": np.ascontiguousarray(bg.reshape(DT, 128).T),
        "ones128": np.ones((1, 128), BF16NP),
    }
    for pc in per_core:
        pc.update(shared)
    return per_core, S_b, S_raw, segs, gnn_idx, hs


def _build_program(S_b, S_raw, segs):
    nc = bacc.Bacc("TRN2", target_bir_lowering=False, debug=False,
                   num_devices=NCORES)
    S_MAX = max(S_b)
    NT_TOT = sum(2 * 128 * s for s in S_b)

    d_nodesT = nc.dram_tensor("nodesT", [NT_TOT], FP8, kind="ExternalInput").ap()
    d_Wn = nc.dram_tensor("Wn", [E, D], FP8, kind="ExternalInput").ap()
    d_Wg = nc.dram_tensor("Wg", [E, D], BF16, kind="ExternalInput").ap()
    d_bn = nc.dram_tensor("bn", [128, DT], F32, kind="ExternalInput").ap()
    d_bg = nc.dram_tensor("bg", [128, DT], F32, kind="ExternalInput").ap()
    d_ones = nc.dram_tensor("ones128", [1, 128], BF16, kind="ExternalInput").ap()
    d_maskf = nc.dram_tensor("maskf", [NB, N, M], F32, kind="ExternalInput").ap()
    d_spr = nc.dram_tensor("spr", [N, NB], F32, kind="ExternalInput").ap()
    d_sgq = nc.dram_tensor("sgq", [128, DT, QPC], BF16, kind="ExternalInput").ap()
    d_scq = nc.dram_tensor("scq", [N, QPC], F32, kind="ExternalInput").ap()
    d_outT = nc.dram_tensor("outT", [D, QPC], F32, kind="ExternalOutput").ap()

    # DRAM scratch
    d_dots = nc.dram_tensor("sc_dots", [NB, 2048], F32).ap()
    d_wa = nc.dram_tensor("sc_wa", [NB, 2048], BF16).ap()
    d_ginv = nc.dram_tensor("sc_ginv", [NB, 1], F32).ap()

    with tile.TileContext(nc) as tc:
        with tc.tile_pool(name="wts", bufs=1) as wts, \
             tc.tile_pool(name="big", bufs=4) as big, \
             tc.tile_pool(name="zp", bufs=10) as zp, \
             tc.tile_pool(name="sml", bufs=6) as sml, \
             tc.tile_pool(name="mep", bufs=24) as mep, \
             tc.tile_pool(name="jnk", bufs=2) as jnk, \
             tc.tile_pool(name="ps", bufs=2, space="PSUM") as ps, \
             tc.tile_pool(name="psd", bufs=1, space="PSUM") as psd, \
             tc.tile_pool(name="psw", bufs=1, space="PSUM") as psw:

            # ---------------- load constants ----------------
            sWn = wts.tile([128, ET, D], FP8)
            nc.sync.dma_start(sWn, d_Wn.rearrange("(t p) r -> p t r", p=128))
            sWg = wts.tile([128, ET, D], BF16)
            nc.sync.dma_start(sWg, d_Wg.rearrange("(t p) r -> p t r", p=128))
            sbn = wts.tile([128, DT], F32)
            nc.sync.dma_start(sbn, d_bn)
            sbg = wts.tile([128, DT], F32)
            nc.sync.dma_start(sbg, d_bg)
            sones_r = wts.tile([1, 128], BF16)
            nc.sync.dma_start(sones_r, d_ones)
            smaskf = wts.tile([N, NB, M], F32)
            nc.sync.dma_start(smaskf, d_maskf.rearrange("b n m -> n b m"))
            spr = wts.tile([N, NB], F32)
            nc.sync.dma_start(spr, d_spr)
            sgq = wts.tile([128, DT, QPC], BF16)
            nc.sync.dma_start(sgq, d_sgq)
            scq = wts.tile([N, QPC], F32)
            nc.sync.dma_start(scq, d_scq)
            zpad = wts.tile([1, 16], BF16)
            nc.vector.memset(zpad, 0.0)

            # ---------------- main loop ----------------
            t_pooled = wts.tile([128, ET, QPC], F32)
            nt_off = 0
            from collections import defaultdict
            partials = defaultdict(list)
            for b in range(NB):
                qi, k = b // K, b % K
                sb = S_b[b]

                t_nt = big.tile([128, ET, S_MAX], FP8, tag="nt")
                nc.sync.dma_start(
                    t_nt[:, :, 0:sb],
                    bass.AP(tensor=d_nodesT.tensor, offset=nt_off,
                            ap=[[sb, 128], [128 * sb, ET], [1, sb]]))
                nt_off += 2 * 128 * sb

                # ---- X = Wn^T @ nt (fp8 DoubleRow), tanh -> z, dots via PE ----
                t_dots = sml.tile([1, S_MAX], F32, tag="dots")
                for a0 in range(0, sb, ACTW):
                    aw = min(ACTW, sb - a0)
                    zs = []
                    for dt_i in range(DT):
                        p_x = ps.tile([128, ACTW], F32, tag="mm")
                        for (p0, pw) in _pieces(aw):
                            nc.tensor.matmul(
                                p_x[:, p0 : p0 + pw],
                                sWn[:, :, dt_i * 128:(dt_i + 1) * 128],
                                t_nt[:, :, a0 + p0 : a0 + p0 + pw],
                                start=True, stop=True, perf_mode=DR)
                        t_z = zp.tile([128, ACTW], BF16, tag="z")
                        nc.scalar.activation(t_z[:, 0:aw], p_x[:, 0:aw],
                                             AF.Tanh, bias=sbn[:, dt_i : dt_i + 1],
                                             scale=1.0)
                        zs.append(t_z)
                    for (p0, pw) in _pieces(aw):
                        p_d = psd.tile([1, PSW], F32, tag="dr")
                        for dt_i in range(DT):
                            nc.tensor.matmul(p_d[0:1, 0:pw],
                                             sgq[:, dt_i, qi : qi + 1],
                                             zs[dt_i][:, p0 : p0 + pw],
                                             start=(dt_i == 0),
                                             stop=(dt_i == DT - 1))
                        nc.vector.tensor_copy(t_dots[0:1, a0 + p0 : a0 + p0 + pw],
                                              p_d[0:1, 0:pw])
                nc.sync.dma_start(d_dots[b : b + 1, 0:sb], t_dots[0:1, 0:sb])

                # ---- softmax on [N, M] grid ----
                t_dbuf = sml.tile([N, M], F32, tag="dbuf")
                nc.gpsimd.tensor_scalar(t_dbuf, smaskf[:, b, :], 0.0,
                                        scq[:, qi : qi + 1],
                                        op0=ALU.mult, op1=ALU.add)
                for (L, r0, cnt, soff) in segs[b]:
                    nc.sync.dma_start(
                        t_dbuf[r0 : r0 + cnt, 0:L],
                        d_dots[b, soff : soff + cnt * L].rearrange("(c l) -> c l", l=L))

                t_gmx = sml.tile([N, 1], F32, tag="gmx")
                nc.vector.tensor_reduce(t_gmx, t_dbuf, axis=AX.X, op=ALU.max)
                t_gnmx = sml.tile([N, 1], F32, tag="gnmx")
                nc.gpsimd.tensor_scalar_mul(t_gnmx, t_gmx, -1.0)
                t_ex = sml.tile([N, M], F32, tag="ex")
                t_rs = sml.tile([N, 1], F32, tag="rs")
                nc.scalar.activation(t_ex, t_dbuf, AF.Exp, bias=t_gnmx, scale=1.0,
                                     accum_out=t_rs)
                t_ri = sml.tile([N, 1], F32, tag="ri")
                nc.vector.reciprocal(t_ri, t_rs)
                t_lg = sml.tile([N, M], F32, tag="lg")
                nc.vector.tensor_scalar(t_lg, t_ex, t_ri, spr[:, b : b + 1],
                                        op0=ALU.mult, op1=ALU.mult)
                t_gl = sml.tile([N, M], F32, tag="gl")
                t_grs = sml.tile([N, 1], F32, tag="grs")
                nc.scalar.activation(t_gl, t_lg, AF.Exp, accum_out=t_grs)
                t_gs = sml.tile([1, 1], F32, tag="gs")
                nc.gpsimd.tensor_reduce(t_gs, t_grs, axis=AX.C, op=ALU.add)
                t_gi = sml.tile([1, 1], F32, tag="gi")
                nc.vector.reciprocal(t_gi, t_gs)
                nc.sync.dma_start(d_ginv[b : b + 1, :], t_gi)
                t_gic = sml.tile([N, 1], F32, tag="gic")
                nc.sync.dma_start(
                    t_gic,
                    bass.AP(tensor=d_ginv.tensor, offset=b, ap=[[0, N], [1, 1]]))
                t_wa = sml.tile([N, M], BF16, tag="wa")
                nc.vector.scalar_tensor_tensor(
                    t_wa, t_gl, t_gic, smaskf[:, b, :],
                    op0=ALU.mult, op1=ALU.mult)

                # gather back to compacted order (+ zero the 16-pad tail)
                for (L, r0, cnt, soff) in segs[b]:
                    nc.sync.dma_start(
                        d_wa[b, soff : soff + cnt * L].rearrange("(c l) -> c l", l=L),
                        t_wa[r0 : r0 + cnt, 0:L])
                if S_raw[b] < sb:
                    nc.sync.dma_start(d_wa[b, S_raw[b] : sb],
                                      zpad[0, 0 : sb - S_raw[b]])
                t_wac = sml.tile([1, S_MAX], BF16, tag="wac")
                nc.sync.dma_start(t_wac[0:1, 0:sb], d_wa[b : b + 1, 0:sb])

                # ---- pass 2: me[e] = sum_s nodesT[e, s] * wa[s] ----
                for (p0, pw) in _pieces(sb):
                    p_w = psw.tile([128, PSW], F32, tag="wb")
                    nc.tensor.matmul(p_w[:, 0:pw], sones_r,
                                     t_wac[0:1, p0 : p0 + pw],
                                     start=True, stop=True)
                    for et in range(ET):
                        t_me = mep.tile([128, 1], F32, tag="me")
                        t_junk = jnk.tile([128, PSW], BF16, tag="junk")
                        nc.vector.scalar_tensor_tensor(
                            out=t_junk[:, 0:pw],
                            in0=t_nt[:, et, p0 : p0 + pw],
                            scalar=1.0,
                            in1=p_w[:, 0:pw],
                            op0=ALU.mult, op1=ALU.mult,
                            accum_out=t_me)
                        partials[(qi, et)].append(t_me)
                if k == K - 1:
                    for et in range(ET):
                        ps_list = partials.pop((qi, et))
                        acc_t = ps_list[0]
                        for i, t in enumerate(ps_list[1:]):
                            is_last = i == len(ps_list) - 2
                            if is_last:
                                dst = t_pooled[:, et, qi : qi + 1]
                            else:
                                dst = mep.tile([128, 1], F32, tag="macc")
                            nc.vector.tensor_tensor(dst, acc_t, t, op=ALU.add)
                            acc_t = dst

            # ---------------- output projection ----------------
            t_plr = wts.tile([128, ET, QPC], BF16)
            nc.vector.tensor_copy(t_plr, t_pooled)
            t_outT = wts.tile([128, DT, QPC], F32)
            for mt in range(DT):
                p_o = ps.tile([128, ACTW], F32, tag="mm")
                for et in range(ET):
                    nc.tensor.matmul(p_o[:, 0:QPC],
                                     sWg[:, et, mt * 128:(mt + 1) * 128],
                                     t_plr[:, et, :],
                                     start=(et == 0), stop=(et == ET - 1))
                nc.scalar.activation(t_outT[:, mt, :], p_o[:, 0:QPC], AF.Tanh,
                                     bias=sbg[:, mt : mt + 1], scale=1.0)
            nc.sync.dma_start(d_outT.rearrange("(t p) q -> p t q", p=128), t_outT)

    nc.compile()
    return nc


_CACHE = {}


def kernel(**inputs) -> np.ndarray:
    per_core, S_b, S_raw, segs, gnn_idx, hs = _host_prep(inputs)
    key = tuple(S_b)
    if key not in _CACHE:
        _CACHE[key] = _build_program(S_b, S_raw, segs)
    nc = _CACHE[key]
    res = run_bass_kernel_spmd(nc, per_core, list(range(NCORES)))
    out = hs.copy()
    for c in range(NCORES):
        rows = res.results[c]["outT"].T      # [QPC, D]
        np.add.at(out, gnn_idx[c * QPC : (c + 1) * QPC], rows)
    return out
